# revision 6
# baseline (speedup 1.0000x reference)
"""Trainium2 Bass kernel for CSSM (Mamba-style 2D selective scan block).

Sharding: 8 cores = 4 batch x 2 d_inner-halves. Each core computes the
full front-end (convs/projections) for its batch element, the selective
scan for its 96 d_inner channels (x 16 states packed into 12 tiles of 128
partition lanes), and a partial output projection. The host sums the two
partial outputs per batch element.

Key structure exploited: the scan recurrence h_t = dA_t*h_{t-1} + dBu_t maps
directly onto the native DVE tensor_tensor_scan instruction (one lane per
(d, n) state pair, time along the free dim). All cross-partition data
movement (broadcasting B/C rows, replicating per-channel quantities into the
packed (d, n) lane layout, and the final sum over n) is done on the tensor
engine via host-precomputed selector matrices with PSUM accumulation.
"""
import sys

sys.path.insert(0, "/opt/trn_rl_repo")

import numpy as np

C = 96            # d_model; also channels per d_inner half
DI = 192          # d_inner
NST = 16          # d_state
DTR = 6           # dt_rank
HH = 64
WW = 64
L = HH * WW       # 4096
T = 512           # L-chunk size
NCH = L // T      # 8
NLANE = C * NST   # 1536 packed (d, n) lanes per core
NJ = NLANE // 128 # 12 lane tiles
PW = WW + 2       # 66: padded row width for the 3x3 conv
G = 68            # left guard of the padded conv buffer
PADLEN = G + (HH + 2) * PW + 68

_CACHE = {}


def _emit(tc, nc, mybir, dram):
    from contextlib import ExitStack

    f32 = mybir.dt.float32
    AF = mybir.ActivationFunctionType
    OP = mybir.AluOpType

    with ExitStack() as ctx:
        consts = ctx.enter_context(tc.tile_pool(name="consts", bufs=1))
        persist = ctx.enter_context(tc.tile_pool(name="persist", bufs=1))

        def cload(name, shape, rearr=None):
            t = consts.tile(list(shape), f32, tag=name)
            src = dram[name]
            if rearr is not None:
                src = src.rearrange(rearr)
            nc.sync.dma_start(t[:], src)
            return t

        wdt_sb = cload("wdt", (32, C))
        bdt_sb = cload("bdt", (C, 1))
        dvec_sb = cload("dvec", (C, 1))
        wout_sb = cload("wout", (C, C))
        b1d_sb = cload("b1d", (C, 2))
        sel_nd = cload("sel_nd", (C, NJ, 128), "j k m -> k j m")
        sel_b = cload("sel_b", (38, NJ, 128), "j k m -> k j m")
        sel_c = cload("sel_c", (38, NJ, 128), "j k m -> k j m")
        sel_v = cload("sel_v", (C, NJ, 128), "j k m -> k j m")
        red_sb = cload("red", (128, NJ, C), "j l m -> l j m")

        x_dbl = persist.tile([38, L], f32, tag="x_dbl")
        sz = persist.tile([C, L], f32, tag="sz")
        xc_a = persist.tile([C, L], f32, tag="xc_a")
        carry = persist.tile([128, NJ], f32, tag="carry")

        # ---------------- front end ----------------
        with (
            tc.tile_pool(name="fbuf", bufs=3) as fbuf,
            tc.tile_pool(name="fw", bufs=1) as fw,
            tc.tile_pool(name="fxc", bufs=1) as fxc,
            tc.tile_pool(name="fchunk", bufs=2) as fchunk,
            tc.tile_pool(name="fps", bufs=4, space="PSUM") as fps,
        ):
            wp_sb = fw.tile([C, DI], f32, tag="wp")
            nc.sync.dma_start(wp_sb[:], dram["wp"])
            wd_sb = fw.tile([C, 18, C], f32, tag="wd")
            nc.sync.dma_start(wd_sb[:], dram["wd"].rearrange("t g k m -> k (t g) m"))
            win_sb = fw.tile([C, 3 * C], f32, tag="win")
            nc.sync.dma_start(win_sb[:], dram["win"])
            w1d_sb = fw.tile([C, 8, C], f32, tag="w1d")
            nc.sync.dma_start(w1d_sb[:], dram["w1d"].rearrange("g t k m -> k (g t) m"))
            wxp_sb = fw.tile([C, 2, 38], f32, tag="wxp")
            nc.sync.dma_start(wxp_sb[:], dram["wxp"].rearrange("g k m -> k g m"))

            xp1 = [fbuf.tile([C, PADLEN], f32, tag="fbuf", name=f"xp1_{g}")
                   for g in range(2)]
            nc.gpsimd.memset(xp1[0][:], 0.0)
            nc.gpsimd.memset(xp1[1][:], 0.0)

            # 1x1 proj: x (96, L) -> xp1 (192, L) written into padded layout
            for c in range(NCH):
                xck = fchunk.tile([C, T], f32, tag="xin")
                nc.sync.dma_start(xck[:], dram["x"][:, c * T:(c + 1) * T])
                for g in range(2):
                    ps = fps.tile([C, T], f32, tag="fps")
                    nc.tensor.matmul(ps[:], wp_sb[:, g * C:(g + 1) * C], xck[:])
                    dst = xp1[g][:, G + (8 * c + 1) * PW + 1: G + (8 * c + 9) * PW + 1]
                    dst = dst.rearrange("p (r w) -> p r w", w=PW)[:, :, 0:WW]
                    nc.scalar.activation(dst, ps.rearrange("p (r w) -> p r w", w=WW),
                                         AF.Copy)

            # 3x3 dconv (192 -> 96 ch), evac to dense xc2
            xc2 = fxc.tile([C, L], f32, tag="xc2")
            for i in range(10):
                rows = 7 if i < 9 else 1
                cols = rows * PW
                base = G + (7 * i + 1) * PW
                ps_full = fps.tile([C, 462], f32, tag="fps", name=f"dconv_{i}")
                ps = ps_full[:, :cols]
                k = 0
                for tap in range(9):
                    dy, dx = tap // 3, tap % 3
                    shift = (dy - 1) * PW + (dx - 1)
                    for g in range(2):
                        nc.tensor.matmul(
                            ps, wd_sb[:, tap * 2 + g, :],
                            xp1[g][:, base + shift: base + shift + cols],
                            start=(k == 0), stop=(k == 17))
                        k += 1
                src = ps.rearrange("p (r w) -> p r w", w=PW)[:, :, 1:65]
                dst = xc2[:, 7 * i * WW: (7 * i + rows) * WW]
                nc.scalar.activation(dst.rearrange("p (r w) -> p r w", w=WW), src,
                                     AF.Copy)

            # in_proj: xc2 -> xin (2 halves, causal-padded) + silu(z_local)
            xinp = [fbuf.tile([C, 3 + L], f32, tag="fbuf", name=f"xinp_{g}")
                    for g in range(2)]
            nc.gpsimd.memset(xinp[0][:, 0:3], 0.0)
            nc.gpsimd.memset(xinp[1][:, 0:3], 0.0)
            for c in range(NCH):
                for g in range(3):
                    ps = fps.tile([C, T], f32, tag="fps")
                    nc.tensor.matmul(ps[:], win_sb[:, g * C:(g + 1) * C],
                                     xc2[:, c * T:(c + 1) * T])
                    if g < 2:
                        nc.scalar.activation(
                            xinp[g][:, 3 + c * T: 3 + (c + 1) * T], ps[:], AF.Copy)
                    else:
                        nc.scalar.activation(sz[:, c * T:(c + 1) * T], ps[:], AF.Silu)

            # causal depthwise conv1d (4 taps) + bias + silu -> xc halves
            xc_b = fbuf.tile([C, L], f32, tag="fbuf")
            for g in range(2):
                dst_all = xc_a if g == 0 else xc_b
                for c in range(NCH):
                    ps = fps.tile([C, T], f32, tag="fps")
                    for k in range(4):
                        nc.tensor.matmul(ps[:], w1d_sb[:, g * 4 + k, :],
                                         xinp[g][:, c * T + k: c * T + k + T],
                                         start=(k == 0), stop=(k == 3))
                    nc.scalar.activation(dst_all[:, c * T:(c + 1) * T], ps[:],
                                         AF.Silu, bias=b1d_sb[:, g:g + 1])

            # x_proj: (38, L) = wxp0 @ xc_a + wxp1 @ xc_b
            for c in range(NCH):
                ps = fps.tile([38, T], f32, tag="fps")
                nc.tensor.matmul(ps[:], wxp_sb[:, 0, :], xc_a[:, c * T:(c + 1) * T],
                                 start=True, stop=False)
                nc.tensor.matmul(ps[:], wxp_sb[:, 1, :], xc_b[:, c * T:(c + 1) * T],
                                 start=False, stop=True)
                nc.scalar.activation(x_dbl[:, c * T:(c + 1) * T], ps[:], AF.Copy)

        # ---------------- scan + tail ----------------
        with (
            tc.tile_pool(name="dl", bufs=2) as dl,
            tc.tile_pool(name="lp", bufs=3) as lp,
            tc.tile_pool(name="hp", bufs=3) as hp,
            tc.tile_pool(name="tl", bufs=2) as tl,
            tc.tile_pool(name="pslam", bufs=1, space="PSUM") as pslam,
            tc.tile_pool(name="psbb", bufs=1, space="PSUM") as psbb,
            tc.tile_pool(name="psrv", bufs=1, space="PSUM") as psrv,
            tc.tile_pool(name="pscc", bufs=2, space="PSUM") as pscc,
            tc.tile_pool(name="psy", bufs=1, space="PSUM") as psy,
            tc.tile_pool(name="pmix", bufs=2, space="PSUM") as pmix,
        ):
            for c in range(NCH):
                cs, ce = c * T, (c + 1) * T
                psD = pmix.tile([C, T], f32, tag="pmix")
                nc.tensor.matmul(psD[:], wdt_sb[:], x_dbl[:32, cs:ce])
                edt = dl.tile([C, T], f32, tag="edt")
                nc.scalar.activation(edt[:], psD[:], AF.Exp, bias=bdt_sb[:])
                delta_c = dl.tile([C, T], f32, tag="delta")
                nc.scalar.activation(delta_c[:], edt[:], AF.Ln, bias=1.0)
                du_c = dl.tile([C, T], f32, tag="du")
                nc.vector.tensor_mul(du_c[:], delta_c[:], xc_a[:, cs:ce])

                yP = psy.tile([C, T], f32, tag="psy")
                for j in range(NJ):
                    lamP = pslam.tile([128, T], f32, tag="pslam")
                    nc.tensor.matmul(lamP[:], sel_nd[:, j, :], delta_c[:])
                    dA = lp.tile([128, T], f32, tag="dA")
                    nc.scalar.activation(dA[:], lamP[:], AF.Exp, scale=-1.0)

                    bbP = psbb.tile([128, T], f32, tag="psbb")
                    nc.tensor.matmul(bbP[:], sel_b[:, j, :], x_dbl[:, cs:ce])
                    bb = lp.tile([128, T], f32, tag="bb")
                    nc.scalar.activation(bb[:], bbP[:], AF.Copy)

                    rvP = psrv.tile([128, T], f32, tag="psrv")
                    nc.tensor.matmul(rvP[:], sel_v[:, j, :], du_c[:])
                    dBu = lp.tile([128, T], f32, tag="dBu")
                    nc.vector.tensor_mul(dBu[:], bb[:], rvP[:])

                    ccP = pscc.tile([128, T], f32, tag="pscc")
                    nc.tensor.matmul(ccP[:], sel_c[:, j, :], x_dbl[:, cs:ce])

                    h = hp.tile([128, T], f32, tag="h")
                    init = 0.0 if c == 0 else carry[:, j:j + 1]
                    nc.vector.tensor_tensor_scan(h[:], dA[:], dBu[:], init,
                                                 OP.mult, OP.add)
                    nc.vector.tensor_copy(carry[:, j:j + 1], h[:, T - 1:T])

                    tmp = lp.tile([128, T], f32, tag="tmp")
                    nc.vector.tensor_mul(tmp[:], h[:], ccP[:])
                    nc.tensor.matmul(yP[:], red_sb[:, j, :], tmp[:],
                                     start=(j == 0), stop=(j == NJ - 1))

                yg = tl.tile([C, T], f32, tag="yg")
                nc.vector.scalar_tensor_tensor(yg[:], xc_a[:, cs:ce], dvec_sb[:, 0:1],
                                               yP[:], OP.mult, OP.add)
                y2 = tl.tile([C, T], f32, tag="y2")
                nc.vector.tensor_mul(y2[:], yg[:], sz[:, cs:ce])
                outP = pmix.tile([C, T], f32, tag="pmix")
                nc.tensor.matmul(outP[:], wout_sb[:], y2[:])
                osb = tl.tile([C, T], f32, tag="osb")
                nc.scalar.activation(osb[:], outP[:], AF.Copy)
                nc.sync.dma_start(dram["out_part"][:, cs:ce], osb[:])


def _build_program():
    from concourse import bacc, tile, mybir

    nc = bacc.Bacc("TRN2", target_bir_lowering=False, debug=False, num_devices=8)
    f32 = mybir.dt.float32

    def din(name, shape):
        return nc.dram_tensor(name, shape, f32, kind="ExternalInput").ap()

    dram = {
        "x": din("x", (C, L)),
        "wp": din("wp", (C, DI)),
        "wd": din("wd", (9, 2, C, C)),
        "win": din("win", (C, 3 * C)),
        "w1d": din("w1d", (2, 4, C, C)),
        "b1d": din("b1d", (C, 2)),
        "wxp": din("wxp", (2, C, 38)),
        "wdt": din("wdt", (32, C)),
        "bdt": din("bdt", (C, 1)),
        "dvec": din("dvec", (C, 1)),
        "wout": din("wout", (C, C)),
        "sel_nd": din("sel_nd", (NJ, C, 128)),
        "sel_b": din("sel_b", (NJ, 38, 128)),
        "sel_c": din("sel_c", (NJ, 38, 128)),
        "sel_v": din("sel_v", (NJ, C, 128)),
        "red": din("red", (NJ, 128, C)),
        "out_part": nc.dram_tensor("out_part", (C, L), f32,
                                   kind="ExternalOutput").ap(),
    }

    with tile.TileContext(nc) as tc:
        _emit(tc, nc, mybir, dram)
    nc.compile()
    return nc


def get_program():
    if "nc" not in _CACHE:
        _CACHE["nc"] = _build_program()
    return _CACHE["nc"]


def make_core_inputs(inputs, b, half):
    perm = np.concatenate([
        np.arange(half * C, half * C + C),
        np.arange((1 - half) * C, (1 - half) * C + C),
    ])
    loc = perm[:C]

    gl = np.arange(NLANE)
    n_g, d_g = gl // C, gl % C

    a = np.exp(np.asarray(inputs["A_log"], np.float64))[loc].astype(np.float32)
    sel_nd = np.zeros((NJ, C, 128), np.float32)
    sel_b = np.zeros((NJ, 38, 128), np.float32)
    sel_c = np.zeros((NJ, 38, 128), np.float32)
    sel_v = np.zeros((NJ, C, 128), np.float32)
    red = np.zeros((NJ, 128, C), np.float32)
    for j in range(NJ):
        m = np.arange(128)
        gg = j * 128 + m
        sel_nd[j, d_g[gg], m] = a[d_g[gg], n_g[gg]]
        sel_b[j, DTR + n_g[gg], m] = 1.0
        sel_c[j, DTR + NST + n_g[gg], m] = 1.0
        sel_v[j, d_g[gg], m] = 1.0
        red[j, m, d_g[gg]] = 1.0

    wd = np.empty((9, 2, C, C), np.float32)
    dw = np.asarray(inputs["dconv_w"], np.float32)   # (96, 192, 3, 3)
    for tap in range(9):
        dy, dx = tap // 3, tap % 3
        for g in range(2):
            wd[tap, g] = dw[:, g * C:(g + 1) * C, dy, dx].T

    w_in = np.asarray(inputs["in_proj_w"], np.float32)
    win = np.concatenate([w_in[perm[:C]].T, w_in[perm[C:]].T,
                          w_in[DI + loc].T], axis=1)

    w1 = np.asarray(inputs["conv1d_w"], np.float32)[perm]   # (192, 4)
    w1d = np.zeros((2, 4, C, C), np.float32)
    for g in range(2):
        for k in range(4):
            np.fill_diagonal(w1d[g, k], w1[g * C:(g + 1) * C, k])
    b1 = np.asarray(inputs["conv1d_b"], np.float32)[perm]
    b1d = np.stack([b1[:C], b1[C:]], axis=1)

    wxp_full = np.asarray(inputs["x_proj_w"], np.float32)[:, perm]  # (38, 192)
    wxp = np.stack([wxp_full[:, :C].T, wxp_full[:, C:].T], axis=0)

    wdt = np.zeros((32, C), np.float32)
    wdt[:DTR] = np.asarray(inputs["dt_proj_w"], np.float32)[loc].T

    return {
        "x": np.ascontiguousarray(
            np.asarray(inputs["x"], np.float32)[b].reshape(C, L)),
        "wp": np.ascontiguousarray(
            np.asarray(inputs["proj_w"], np.float32)[:, :, 0, 0].T),
        "wd": wd,
        "win": np.ascontiguousarray(win),
        "w1d": w1d,
        "b1d": np.ascontiguousarray(b1d),
        "wxp": np.ascontiguousarray(wxp),
        "wdt": wdt,
        "bdt": np.asarray(inputs["dt_proj_b"], np.float32)[loc, None],
        "dvec": np.asarray(inputs["D"], np.float32)[loc, None],
        "wout": np.ascontiguousarray(
            np.asarray(inputs["out_proj_w"], np.float32)[:, loc].T),
        "sel_nd": sel_nd,
        "sel_b": sel_b,
        "sel_c": sel_c,
        "sel_v": sel_v,
        "red": red,
    }


def kernel(**inputs):
    from concourse import bass_utils

    nc = get_program()
    in_maps = [make_core_inputs(inputs, b, half)
               for b in range(4) for half in range(2)]
    res = bass_utils.run_bass_kernel_spmd(nc, in_maps, core_ids=list(range(8)))
    out = np.zeros((4, C, L), np.float32)
    for b in range(4):
        out[b] = res.results[2 * b]["out_part"] + res.results[2 * b + 1]["out_part"]
    return out.reshape(4, C, HH, WW)
